# revision 1
# baseline (speedup 1.0000x reference)
"""MoE CouncilLayer kernel for 8x TRN2 NeuronCores (expert-parallel).

Problem (all-expert MoE, B=2, T=1024, C=768, E=32, H=3072):
    gates = softmax(x @ gate_w + gate_b)                     # [N, E]
    h     = gelu(einsum('nc,ech->neh', x, w1) + b1)          # [N, E, H]
    y     = einsum('neh,ehc->nec', h, w2) + b2               # [N, E, C]
    out   = einsum('ne,nec->nc', gates, y)                   # [N, C]

Sharding: expert-parallel, 4 experts per core; x replicated. Each core
computes its 4 experts' gate-weighted partial sum; host adds the 8
partials (the all-reduce is a cheap 6 MB/core host-side sum).

Per-core layout is feature-major (activations stored [feature, token]):
    mm1: psum[h_blk, t] += w1[c_blk, h_blk].T @ xT[c_blk, t]   (fp16)
    hg  = gelu(psum + b1) * gate_bcast                         (fp16)
    mm2: psum[c_blk, t] += w2[h_blk, c_blk].T @ hg[h_blk, t]   (fp16)
         (+ rank-4 matmul b2T.T @ g_localT folded into the same psum
          accumulation, so b2's gate-weighted contribution is free)
b2's gate-weighted term rides the cross-expert accumulation as fused
DVE scalar_tensor_tensor multiply-adds (per-partition b2 scalar x
token-broadcast gate), keeping it off the PE entirely.

Gates are computed on-device (fp16 PE matmuls for logits, fp32 ACT exp,
one fp16 PE ones-matmul per token chunk to sum over the expert
partition axis, DVE reciprocal); the [128, N] per-expert gate broadcast
is a partition-broadcast DMA through a DRAM bounce buffer. Gate columns
are permuted host-side so every core's 4 local experts sit at columns
0..3 (keeps the SPMD program core-agnostic).

Head scheduling exploits PE's in-order execution: the first four
h-blocks of mm1 are traced cc-OUTER across all 8 psum banks so PE
consumes each x chunk as it lands (DMAs sequenced b1 -> interleaved
w1/x-half0 groups -> gate consts -> prefetched second w1 tile ->
x-half1), saturating PE from ~4us. The gate softmax is traced behind
it and drains on ACT/DVE under the matmul stream.

Cost model (InstructionCostModel timeline, one core): ~997.4 us total,
PE busy 987.9 us of which 983.0 us is the irreducible 4608 fp16
[128x128]@[128x512] matmul stream; idle is ~4us head (first-DMA
latency) + ~4.7us tail (drain + out-DMA + barrier) -> 98.6% of the
fp16 PE roofline. The gate logits and softmax denominators are
col-tiled (tile_position 32-column groups, each group in its own psum
bank at partition offset 32*t4) so token-chunk groups run concurrently
on the PE sub-arrays.
"""

import numpy as np

import concourse.tile as tile
from concourse import bacc, mybir
from concourse.bass_utils import run_bass_kernel_spmd

# Problem dims (hardcoded per harness contract)
B, T, C, E, H = 2, 1024, 768, 32, 3072
N = B * T  # 2048 tokens
NCORES = 8
EL = E // NCORES  # 4 local experts
CB = C // 128  # 6 c-blocks
HB = H // 128  # 24 h-blocks
TCG = 2  # token groups (1024 each)
TG = N // TCG  # 1024
TI = TG // 512  # 512-token chunks per group

F16 = mybir.dt.float16
F32 = mybir.dt.float32
AF = mybir.ActivationFunctionType

_CACHED_NC = None


def build_nc(act=AF.Gelu):
    nc = bacc.Bacc(trn_type="TRN2")

    xT16_d = nc.dram_tensor("xT16", [C, N], F16, kind="ExternalInput")
    gw_d = nc.dram_tensor("gw", [C, E], F16, kind="ExternalInput")
    gb_d = nc.dram_tensor("gb", [E, 1], F32, kind="ExternalInput")
    ones_d = nc.dram_tensor("ones32", [E, EL], F16, kind="ExternalInput")
    w1_d = nc.dram_tensor("w1", [EL, C, H], F16, kind="ExternalInput")
    b1_d = nc.dram_tensor("b1", [128, EL, HB], F32, kind="ExternalInput")
    w2_d = nc.dram_tensor("w2", [EL, H, C], F16, kind="ExternalInput")
    b2P_d = nc.dram_tensor("b2P", [128, EL, CB], F32, kind="ExternalInput")
    outT_d = nc.dram_tensor("outT", [C, N], F32, kind="ExternalOutput")

    with tile.TileContext(nc) as tc:
        with (
            tc.tile_pool(name="const", bufs=1) as cp,
            tc.tile_pool(name="stream", bufs=1) as sp,
            tc.tile_pool(name="psum", bufs=1, space="PSUM") as pp,
            tc.tile_pool(name="dram", bufs=1, space="DRAM") as dp,
        ):
            # --- resident tiles ---
            xT16_sb = cp.tile([128, CB, N], F16)
            gw_sb = cp.tile([128, CB, E], F16)
            gb_sb = cp.tile([E, 1], F32)
            ones_sb = cp.tile([E, EL], F16)
            b1_sb = cp.tile([128, EL, HB], F32)
            b2P_sb = cp.tile([128, EL, CB], F32)
            expT_sb = cp.tile([E, N], F32)
            expT16_sb = cp.tile([E, N], F16)
            g_bcast_sb = cp.tile([128, EL, N], F16)
            g_localT_sb = cp.tile([EL, N], F16)

            # DMA issue order = arrival order on the queue; sequence it so
            # the specially-traced first mm1 block (which only needs b1, its
            # w1 tile, and the FIRST-half token columns of each x chunk) can
            # start ~5us in and then stays ahead of the arrival stream. The
            # gate constants ride between the two x halves; b2T (needed at
            # ~140us) goes last.
            w1t_first = sp.tile([128, CB, 512], F16, tag="w1", bufs=3, name="w1t")
            w1f_ap = w1_d[0, :, :].rearrange("(cc p) h -> p cc h", p=128)[:, :, 0:512]
            x_ap = xT16_d[:, :].rearrange("(cc p) t -> p cc t", p=128)
            # first two groups are single-cc so the opening matmuls fire as
            # early as possible; later groups pair up to amortize DMA overhead.
            # b1 (needed by the first gelu ~7us in) rides after the first group.
            ccs = [slice(0, 1), slice(1, 2), slice(2, 4), slice(4, 6)]
            for ci, cs in enumerate(ccs):
                nc.sync.dma_start(w1t_first[:, cs, :], w1f_ap[:, cs, :])
                nc.sync.dma_start(xT16_sb[:, cs, 0:TG], x_ap[:, cs, 0:TG])
                if ci == 0:
                    nc.sync.dma_start(b1_sb, b1_d[:, :, :])
            nc.sync.dma_start(gw_sb, gw_d[:, :].rearrange("(cc p) e -> p cc e", p=128))
            nc.sync.dma_start(gb_sb, gb_d[:, :])
            nc.sync.dma_start(ones_sb, ones_d[:, :])
            # prefetch e0's second w1 tile ahead of the bulk x second half so
            # mm1 hbg1 isn't gated on the 1.5MB transfer in front of it
            w1t_second = sp.tile([128, CB, 512], F16, tag="w1", bufs=3, name="w1t")
            nc.sync.dma_start(
                w1t_second,
                w1_d[0, :, :].rearrange("(cc p) h -> p cc h", p=128)[:, :, 512:1024],
            )
            nc.sync.dma_start(
                xT16_sb[:, :, TG:N],
                xT16_d[:, :].rearrange("(cc p) t -> p cc t", p=128)[:, :, TG:N],
            )
            nc.sync.dma_start(b2P_sb, b2P_d[:, :, :])

            def emit_softmax():
                # gate logits + exp, denominator, reciprocal, local gates;
                # then broadcast across partitions via a DRAM bounce.
                # lg/dn borrow tag-"y" psum slots (mm2 doesn't need them until
                # ~130us in; these drain by ~15us).
                # each token-chunk logit group gets its own psum bank but is
                # written at partition offset 32*t4, issued in t4-pairs per cc
                # so the two matmuls of a pair run concurrently in different
                # 32-column groups of the PE array (2x col-tiling). Each group
                # has its own start/stop: the has_written zero regions are
                # per-partition-range, so the four groups in one bank are
                # independent accumulation groups.
                lgs = [
                    pp.tile([128, 512], F32, tag="y", bufs=4, name="lg")
                    for _ in range(4)
                ]
                for pair in range(2):
                    for cc in range(CB):
                        for t4 in (2 * pair, 2 * pair + 1):
                            nc.tensor.matmul(
                                lgs[t4][32 * t4 : 32 * (t4 + 1), :],
                                gw_sb[:, cc, :],
                                xT16_sb[:, cc, t4 * 512 : (t4 + 1) * 512],
                                start=(cc == 0),
                                stop=(cc == CB - 1),
                                tile_position=(0, 32 * t4),
                            )
                for t4 in range(N // 512):
                    ts = slice(t4 * 512, (t4 + 1) * 512)
                    lgs4 = lgs[t4][32 * t4 : 32 * (t4 + 1), :]
                    nc.scalar.activation(
                        expT_sb[:, ts], lgs4, AF.Exp, bias=gb_sb, scale=1.0
                    )
                    nc.scalar.activation(
                        expT16_sb[:, ts], lgs4, AF.Exp, bias=gb_sb, scale=1.0
                    )
                # denominators in a second pass: by the time PE reaches these,
                # the exps have drained on ACT, so no per-t4 PE stall. fp16
                # rhs makes them 1 cyc/row (rounding averages out over the
                # 32-term sum), and they are col-tiled like the logits so all
                # four run concurrently.
                dns = [
                    pp.tile([128, 512], F32, tag="y", bufs=4, name="dn")
                    for _ in range(4)
                ]
                for t4 in range(N // 512):
                    ts = slice(t4 * 512, (t4 + 1) * 512)
                    nc.tensor.matmul(
                        dns[t4][32 * t4 : 32 * t4 + EL, :],
                        ones_sb[:, :],
                        expT16_sb[:, ts],
                        start=True,
                        stop=True,
                        tile_position=(0, 32 * t4),
                    )
                for t4 in range(N // 512):
                    ts = slice(t4 * 512, (t4 + 1) * 512)
                    rc = sp.tile([EL, 512], F32, tag="recip", bufs=2, name="rc")
                    nc.vector.reciprocal(rc, dns[t4][32 * t4 : 32 * t4 + EL, :])
                    nc.vector.tensor_mul(g_localT_sb[:, ts], expT_sb[0:EL, ts], rc)
                g_dram = dp.tile([EL, N], F16, name="g_dram")
                nc.sync.dma_start(g_dram, g_localT_sb[:, :])
                for j in range(EL):
                    nc.sync.dma_start(
                        g_bcast_sb[:, j, :],
                        g_dram[j : j + 1, :].to_broadcast((128, N)),
                    )

            def emit_gelu(tg, e, hb, hps, hg):
                for ti in range(TI):
                    lts = slice(ti * 512, (ti + 1) * 512)
                    nc.scalar.activation(
                        hg[:, hb, lts],
                        hps[ti],
                        act,
                        bias=b1_sb[:, e, hb : hb + 1],
                        scale=1.0,
                    )

            def emit_scale(tg, e, hb, hg):
                # in-place gate scale; must be traced after the g_bcast DMAs
                # so Tile sees the RAW dependency
                for ti in range(TI):
                    gts = slice(tg * TG + ti * 512, tg * TG + (ti + 1) * 512)
                    lts = slice(ti * 512, (ti + 1) * 512)
                    nc.vector.tensor_mul(
                        hg[:, hb, lts],
                        hg[:, hb, lts],
                        g_bcast_sb[:, e, gts],
                    )

            def emit_gelu_scale(tg, e, hb, hps, hg):
                emit_gelu(tg, e, hb, hps, hg)
                emit_scale(tg, e, hb, hg)

            def emit_mm1_first(hg):
                # first 4 h-blocks of (tg0, e0), traced cc-OUTER across all 8
                # psum banks: PE's in-order stream then consumes each arriving
                # x chunk immediately (8 matmuls = 1.7us per 512KB chunk vs
                # ~1.4us DMA cadence) instead of stalling on the last chunk
                # of an accumulation group.
                hps8 = [
                    [
                        pp.tile(
                            [128, 512],
                            F32,
                            tag=("h" if hbi < 2 else "y"),
                            bufs=4,
                            name="hps",
                        )
                        for _ in range(TI)
                    ]
                    for hbi in range(4)
                ]
                for cc in range(CB):
                    for ti in range(TI):
                        for hbi in range(4):
                            nc.tensor.matmul(
                                hps8[hbi][ti],
                                w1t_first[:, cc, hbi * 128 : (hbi + 1) * 128],
                                xT16_sb[:, cc, ti * 512 : (ti + 1) * 512],
                                start=(cc == 0),
                                stop=(cc == CB - 1),
                            )
                for hbi in range(4):
                    emit_gelu(0, 0, hbi, hps8[hbi], hg)

            def emit_mm1(tg, e, hg, hbg_start=0, gelu_only=False):
                # mm1: h = gelu(w1.T @ xT + b1) * g
                for hbg in range(hbg_start, HB // 4):
                    if tg == 0 and e == 0 and hbg == 1:
                        w1t = w1t_second
                    else:
                        w1t = sp.tile([128, CB, 512], F16, tag="w1", bufs=3, name="w1t")
                        nc.sync.dma_start(
                            w1t,
                            w1_d[e, :, :].rearrange("(cc p) h -> p cc h", p=128)[
                                :, :, hbg * 512 : (hbg + 1) * 512
                            ],
                        )
                    for hbi in range(4):
                        hb = hbg * 4 + hbi
                        hps = [
                            pp.tile([128, 512], F32, tag="h", bufs=4, name="hps")
                            for _ in range(TI)
                        ]
                        for cc in range(CB):
                            for ti in range(TI):
                                gts = slice(
                                    tg * TG + ti * 512, tg * TG + (ti + 1) * 512
                                )
                                nc.tensor.matmul(
                                    hps[ti],
                                    w1t[:, cc, hbi * 128 : (hbi + 1) * 128],
                                    xT16_sb[:, cc, gts],
                                    start=(cc == 0),
                                    stop=(cc == CB - 1),
                                )
                        if gelu_only:
                            emit_gelu(tg, e, hb, hps, hg)
                        else:
                            emit_gelu_scale(tg, e, hb, hps, hg)

            def emit_mm2(tg, e, hg, yac):
                # mm2: y_psum = b2T.T @ g_localT + sum_hb w2.T @ hg
                for cb in range(CB):
                    w2t = sp.tile([128, HB, 128], F16, tag="w2", bufs=3, name="w2t")
                    nc.sync.dma_start(
                        w2t,
                        w2_d[e, :, :].rearrange("(hb p) c -> p hb c", p=128)[
                            :, :, cb * 128 : (cb + 1) * 128
                        ],
                    )
                    yps = [
                        pp.tile([128, 512], F32, tag="y", bufs=4, name="yps")
                        for _ in range(TI)
                    ]
                    for hb in range(HB):
                        for ti in range(TI):
                            lts = slice(ti * 512, (ti + 1) * 512)
                            nc.tensor.matmul(
                                yps[ti],
                                w2t[:, hb, :],
                                hg[:, hb, lts],
                                start=(hb == 0),
                                stop=(hb == HB - 1),
                            )
                    for ti in range(TI):
                        gts = slice(tg * TG + ti * 512, tg * TG + (ti + 1) * 512)
                        lts = slice(ti * 512, (ti + 1) * 512)
                        if e == 0:
                            # yac = g_0*b2_0 + psum, then += g_j*b2_j for the
                            # other local experts: b2's gate-weighted term via
                            # fused DVE multiply-adds (per-partition scalar
                            # b2P x token-broadcast gate), off the PE
                            nc.vector.scalar_tensor_tensor(
                                out=yac[:, cb, lts],
                                in0=g_bcast_sb[:, 0, gts],
                                scalar=b2P_sb[:, 0, cb : cb + 1],
                                in1=yps[ti],
                                op0=mybir.AluOpType.mult,
                                op1=mybir.AluOpType.add,
                            )
                            for j in range(1, EL):
                                nc.vector.scalar_tensor_tensor(
                                    out=yac[:, cb, lts],
                                    in0=g_bcast_sb[:, j, gts],
                                    scalar=b2P_sb[:, j, cb : cb + 1],
                                    in1=yac[:, cb, lts],
                                    op0=mybir.AluOpType.mult,
                                    op1=mybir.AluOpType.add,
                                )
                        else:
                            nc.vector.tensor_add(
                                yac[:, cb, lts], yps[ti], yac[:, cb, lts]
                            )

            # --- main. Trace order = PE order: the special first-hbg block
            # (fills the x-arrival window), then the gate prologue (dense,
            # data all present; its DVE/DMA tail overlaps the next mm1), then
            # the rest of the expert stream.
            for tg in range(TCG):
                hg = sp.tile([128, HB, TG], F16, tag="hg", bufs=1, name="hg")
                yac = sp.tile([128, CB, TG], F32, tag="yacc", bufs=1, name="yac")
                for e in range(EL):
                    if tg == 0 and e == 0:
                        emit_mm1_first(hg)
                        emit_softmax()
                        for hbi in range(4):
                            emit_scale(0, 0, hbi, hg)
                        emit_mm1(tg, e, hg, hbg_start=1)
                    else:
                        emit_mm1(tg, e, hg)
                    emit_mm2(tg, e, hg, yac)
                for cb in range(CB):
                    for ti in range(TI):
                        nc.sync.dma_start(
                            outT_d[
                                cb * 128 : (cb + 1) * 128,
                                tg * TG + ti * 512 : tg * TG + (ti + 1) * 512,
                            ],
                            yac[:, cb, ti * 512 : (ti + 1) * 512],
                        )

    nc.compile()
    return nc


def _get_nc():
    global _CACHED_NC
    if _CACHED_NC is None:
        _CACHED_NC = build_nc()
    return _CACHED_NC


def make_in_maps(x, gate_w, gate_b, w1, b1, w2, b2):
    x = np.asarray(x, np.float32)
    gate_w = np.asarray(gate_w, np.float32)
    gate_b = np.asarray(gate_b, np.float32)
    w1 = np.asarray(w1, np.float32)
    b1 = np.asarray(b1, np.float32)
    w2 = np.asarray(w2, np.float32)
    b2 = np.asarray(b2, np.float32)

    xT16 = np.ascontiguousarray(x.reshape(N, C).T).astype(np.float16)
    w1_16 = w1.astype(np.float16)
    w2_16 = w2.astype(np.float16)

    ones32 = np.ones((E, EL), np.float16)

    in_maps = []
    for i in range(NCORES):
        lo, hi = EL * i, EL * (i + 1)
        perm = list(range(lo, hi)) + [e for e in range(E) if not (lo <= e < hi)]
        in_maps.append(
            {
                "xT16": xT16,
                "gw": np.ascontiguousarray(gate_w[:, perm]).astype(np.float16),
                "gb": np.ascontiguousarray(gate_b[perm]).reshape(E, 1),
                "ones32": ones32,
                "w1": w1_16[lo:hi],
                "b1": np.ascontiguousarray(
                    b1[lo:hi].reshape(EL, HB, 128).transpose(2, 0, 1)
                ),
                "w2": w2_16[lo:hi],
                "b2P": np.ascontiguousarray(
                    b2[lo:hi].reshape(EL, CB, 128).transpose(2, 0, 1)
                ),
            }
        )
    return in_maps


def kernel(x, gate_w, gate_b, w1, b1, w2, b2, _trace=False, _tmpdir=None):
    nc = _get_nc()
    in_maps = make_in_maps(x, gate_w, gate_b, w1, b1, w2, b2)
    res = run_bass_kernel_spmd(
        nc,
        in_maps,
        core_ids=list(range(NCORES)),
        trace=_trace,
        tmpdir=_tmpdir,
    )
    acc = res.results[0]["outT"].astype(np.float64)
    for r in res.results[1:]:
        acc += r["outT"]
    out = acc.T.reshape(B, T, C).astype(np.float32)
    if _trace:
        kernel._last_results = res
    return out



# revision 5
# speedup vs baseline: 1.2668x; 1.2668x over previous
"""MoE CouncilLayer kernel for 8x TRN2 NeuronCores (expert-parallel, fp8).

Problem (all-expert MoE, B=2, T=1024, C=768, E=32, H=3072):
    gates = softmax(x @ gate_w + gate_b)                     # [N, E]
    h     = gelu(einsum('nc,ech->neh', x, w1) + b1)          # [N, E, H]
    y     = einsum('neh,ehc->nec', h, w2) + b2               # [N, E, C]
    out   = einsum('ne,nec->nc', gates, y)                   # [N, C]

Sharding: expert-parallel, 4 experts per core; x replicated. Each core
computes its 4 experts' gate-weighted partial sum; host adds the 8
partials.

All matmuls run as fp8e4 (e4m3) DoubleRow matmuls: each instruction
contracts two K=128 blocks at 0.5 cycles per output column - 4x the
fp16 MAC rate. Accuracy is recovered with Dekker-style 2-term fp8
splits of every operand (hi = q8(a), lo = q8(a - hi)) and a 3-product
scheme per logical matmul (hi*hi + lo*hi + hi*lo; the lo*lo term is
dropped), all accumulating in one fp32 PSUM group:
    mm1: h_psum = (x_hi + x_lo) @ w1_hi + x_hi @ w1_lo      (18 K128
         products per [128h x 512t] tile = 9 DoubleRow matmuls)
    mm2: y_psum = (hg_hi + hg_lo) @ w2_hi + hg_hi @ w2_lo
Weights are pre-scaled on the host before quantization (w1 x64,
w2 x128) so their sigma sits mid-range in e4m3 instead of at the
subnormal floor; the inverse scales fold into the ACT gelu input scale
and the gate constants (softmax 'ones' weights = 128 so the on-device
gates come out as g/128, cancelling mm2's x128 psum scale). Measured
end-to-end rel RMS error of this scheme is ~2e-3 (budget 2e-2); PE
time is 6/8 of the fp16 stream = ~740us vs the 983us fp16 floor.

Per-core dataflow per (token-group, expert):
    mm1 DR stream -> psum; ACT pass1 gelu->fp16 scratch, ACT pass2
    gelu->fp8 (hg_hi); DVE subtract -> hg_lo fp8. mm2 DR stream ->
    psum; Pool engine folds (psum + 128*b2) * g_bcast into the fp32
    yac accumulator (scalar_tensor_tensor + tensor_add), keeping DVE
    free for the lo-extraction.

Gates are computed on-device: logits via fp8 DoubleRow Dekker matmuls
(gw_hi/gw_lo x64 host-scaled, 3-product scheme -> ~0.1% logit error),
col-tiled per 512-token chunk in borrowed mm2 psum banks; fp32 ACT exp
(input scale 1/64, bias gate_b), fp16 ones-matmul denominators, DVE
reciprocal; [128, N] per-expert gate broadcast via a DRAM bounce.
Gate columns are permuted host-side so each core's 4 local experts sit
at columns 0..3 (keeps the SPMD program core-agnostic).

Head scheduling: the first four h-blocks of mm1 are traced
stream-outer/cc-outer across all 8 psum banks so PE consumes each
arriving x_hi/x_lo/w1_lo chunk as it lands; the gate softmax is traced
behind them and drains on ACT/DVE under the matmul stream.
"""

import numpy as np
import ml_dtypes

import concourse.tile as tile
from concourse import bacc, mybir
from concourse.bass_utils import run_bass_kernel_spmd

# Problem dims (hardcoded per harness contract)
B, T, C, E, H = 2, 1024, 768, 32, 3072
N = B * T  # 2048 tokens
NCORES = 8
EL = E // NCORES  # 4 local experts
CB = C // 128  # 6 c-blocks
HB = H // 128  # 24 h-blocks
CP = CB // 2  # 3 cc-pairs (DoubleRow K pairs)
HP = HB // 2  # 12 hb-pairs
TCG = 2  # token groups (1024 each)
TG = N // TCG  # 1024
TI = TG // 512  # 512-token chunks per group

S1 = 64.0  # host pre-scale on w1 (and gate_w) before fp8 quantization
S2 = 128.0  # host pre-scale on w2; also folded into softmax denominators

F8 = mybir.dt.float8e4
F16 = mybir.dt.float16
F32 = mybir.dt.float32
AF = mybir.ActivationFunctionType
DR = mybir.MatmulPerfMode.DoubleRow
NP8 = ml_dtypes.float8_e4m3

_CACHED_NC = None


def build_nc(act=AF.Gelu):
    nc = bacc.Bacc(trn_type="TRN2")

    xh_d = nc.dram_tensor("xh", [C, N], F8, kind="ExternalInput")
    xl_d = nc.dram_tensor("xl", [C, N], F8, kind="ExternalInput")
    gwh_d = nc.dram_tensor("gwh", [C, E], F8, kind="ExternalInput")
    gwl_d = nc.dram_tensor("gwl", [C, E], F8, kind="ExternalInput")
    gb_d = nc.dram_tensor("gb", [E, 1], F32, kind="ExternalInput")
    ones_d = nc.dram_tensor("ones32", [E, EL], F16, kind="ExternalInput")
    w1h_d = nc.dram_tensor("w1h", [EL, C, H], F8, kind="ExternalInput")
    w1l_d = nc.dram_tensor("w1l", [EL, C, H], F8, kind="ExternalInput")
    b1_d = nc.dram_tensor("b1", [128, EL, HB], F32, kind="ExternalInput")
    w2h_d = nc.dram_tensor("w2h", [EL, H, C], F8, kind="ExternalInput")
    w2l_d = nc.dram_tensor("w2l", [EL, H, C], F8, kind="ExternalInput")
    b2P_d = nc.dram_tensor("b2P", [128, EL, CB], F32, kind="ExternalInput")
    outT_d = nc.dram_tensor("outT", [C, N], F32, kind="ExternalOutput")

    def w1_ap(dram, e, hs):
        return dram[e, :, :].rearrange("(cc p) h -> p cc h", p=128)[:, :, hs]

    def x_ap(dram):
        return dram[:, :].rearrange("(cc p) t -> p cc t", p=128)

    with tile.TileContext(nc) as tc:
        with (
            tc.tile_pool(name="const", bufs=1) as cp,
            tc.tile_pool(name="stream", bufs=1) as sp,
            tc.tile_pool(name="psum", bufs=1, space="PSUM") as pp,
            tc.tile_pool(name="dram", bufs=1, space="DRAM") as dp,
        ):
            # --- resident tiles ---
            xh_sb = cp.tile([128, CB, N], F8)
            xl_sb = cp.tile([128, CB, N], F8)
            gwh_sb = cp.tile([128, CB, E], F8)
            gwl_sb = cp.tile([128, CB, E], F8)
            gb_sb = cp.tile([E, 1], F32)
            ones_sb = cp.tile([E, EL], F16)
            b1_sb = cp.tile([128, EL, HB], F32)
            b2P_sb = cp.tile([128, EL, CB], F32)
            expT_sb = cp.tile([E, N], F32)
            expT16_sb = cp.tile([E, N], F16)
            g_bcast_sb = cp.tile([128, EL, N], F16)
            g_localT_sb = cp.tile([EL, N], F16)

            # DMA issue order = arrival order. The specially-traced first
            # mm1 block consumes, in order: w1h tile + x_hi (stream A,
            # cc-outer), x_lo (stream B), w1l tile (stream C). b1 rides
            # after the first x_hi chunk (needed by the first gelu); gate
            # consts next (softmax is traced right behind block 0); then
            # the second w1h/w1l tiles and the x halves for token-group 1.
            w1h_first = sp.tile([128, CB, 512], F8, tag="w1", bufs=4, name="w1t")
            w1l_first = sp.tile([128, CB, 512], F8, tag="w1", bufs=4, name="w1t")
            ccs = [slice(0, 2), slice(2, 4), slice(4, 6)]
            for ci, cs in enumerate(ccs):
                nc.sync.dma_start(w1h_first[:, cs, :], w1_ap(w1h_d, 0, slice(0, 512))[:, cs, :])
                nc.sync.dma_start(xh_sb[:, cs, 0:TG], x_ap(xh_d)[:, cs, 0:TG])
                if ci == 0:
                    nc.sync.dma_start(b1_sb, b1_d[:, :, :])
            nc.sync.dma_start(xl_sb[:, :, 0:TG], x_ap(xl_d)[:, :, 0:TG])
            nc.sync.dma_start(w1l_first, w1_ap(w1l_d, 0, slice(0, 512)))
            nc.sync.dma_start(gwh_sb, gwh_d[:, :].rearrange("(cc p) e -> p cc e", p=128))
            nc.sync.dma_start(gwl_sb, gwl_d[:, :].rearrange("(cc p) e -> p cc e", p=128))
            nc.sync.dma_start(gb_sb, gb_d[:, :])
            nc.sync.dma_start(ones_sb, ones_d[:, :])
            # prefetch e0's second w1 tile pair ahead of the bulk second-half
            # x transfer so mm1 hbg1 isn't gated on it
            w1h_second = sp.tile([128, CB, 512], F8, tag="w1", bufs=4, name="w1t")
            w1l_second = sp.tile([128, CB, 512], F8, tag="w1", bufs=4, name="w1t")
            nc.sync.dma_start(w1h_second, w1_ap(w1h_d, 0, slice(512, 1024)))
            nc.sync.dma_start(w1l_second, w1_ap(w1l_d, 0, slice(512, 1024)))
            nc.sync.dma_start(xh_sb[:, :, TG:N], x_ap(xh_d)[:, :, TG:N])
            nc.sync.dma_start(xl_sb[:, :, TG:N], x_ap(xl_d)[:, :, TG:N])
            nc.sync.dma_start(b2P_sb, b2P_d[:, :, :])

            def emit_softmax():
                # logits via fp8 DR Dekker matmuls into borrowed tag-"y"
                # psum slots (mm2 doesn't need them until ~130us in). Each
                # 512-token chunk gets its own psum bank at partition
                # offset 32*t4 (col-tiled so chunk groups run concurrently
                # on PE sub-arrays).
                lgs = [
                    pp.tile([128, 512], F32, tag="y", bufs=4, name="lg")
                    for _ in range(4)
                ]
                for t4 in range(4):
                    ts = slice(t4 * 512, (t4 + 1) * 512)
                    out = lgs[t4][0:32, :]
                    n_st = 3 * CP
                    i = 0
                    for cpi in range(CP):
                        cs = slice(2 * cpi, 2 * cpi + 2)
                        for gw_t, x_t in (
                            (gwh_sb, xh_sb),
                            (gwh_sb, xl_sb),
                            (gwl_sb, xh_sb),
                        ):
                            nc.tensor.matmul(
                                out,
                                gw_t[:, cs, :],
                                x_t[:, cs, ts],
                                start=(i == 0),
                                stop=(i == n_st - 1),
                                perf_mode=DR,
                            )
                            i += 1
                for t4 in range(N // 512):
                    ts = slice(t4 * 512, (t4 + 1) * 512)
                    lgs4 = lgs[t4][0:32, :]
                    nc.scalar.activation(
                        expT_sb[:, ts], lgs4, AF.Exp, bias=gb_sb, scale=1.0 / S1
                    )
                    nc.scalar.activation(
                        expT16_sb[:, ts], lgs4, AF.Exp, bias=gb_sb, scale=1.0 / S1
                    )
                # denominators (x S2 via the ones constant): second pass so
                # the exps have drained on ACT by the time PE reaches these
                dns = [
                    pp.tile([128, 512], F32, tag="y", bufs=4, name="dn")
                    for _ in range(4)
                ]
                for t4 in range(N // 512):
                    ts = slice(t4 * 512, (t4 + 1) * 512)
                    nc.tensor.matmul(
                        dns[t4][32 * t4 : 32 * t4 + EL, :],
                        ones_sb[:, :],
                        expT16_sb[:, ts],
                        start=True,
                        stop=True,
                        tile_position=(0, 32 * t4),
                    )
                for t4 in range(N // 512):
                    ts = slice(t4 * 512, (t4 + 1) * 512)
                    rc = sp.tile([EL, 512], F32, tag="recip", bufs=2, name="rc")
                    nc.vector.reciprocal(rc, dns[t4][32 * t4 : 32 * t4 + EL, :])
                    nc.vector.tensor_mul(g_localT_sb[:, ts], expT_sb[0:EL, ts], rc)
                g_dram = dp.tile([EL, N], F16, name="g_dram")
                nc.sync.dma_start(g_dram, g_localT_sb[:, :])
                for j in range(EL):
                    nc.sync.dma_start(
                        g_bcast_sb[:, j, :],
                        g_dram[j : j + 1, :].to_broadcast((128, N)),
                    )

            def emit_gelu_split(e, hb, hps, hgh, hgl):
                # psum -> fp16 scratch + fp8 hi (ACT), fp8 lo (DVE)
                for ti in range(TI):
                    lts = slice(ti * 512, (ti + 1) * 512)
                    hf = sp.tile([128, 512], F16, tag="hf", bufs=4, name="hf")
                    bias = b1_sb[:, e, hb : hb + 1]
                    nc.scalar.activation(hf, hps[ti], act, bias=bias, scale=1.0 / S1)
                    nc.scalar.activation(
                        hgh[:, hb, lts], hps[ti], act, bias=bias, scale=1.0 / S1
                    )
                    nc.vector.tensor_tensor(
                        hgl[:, hb, lts], hf, hgh[:, hb, lts], mybir.AluOpType.subtract
                    )

            def mm1_streams(w1h_t, w1l_t, xsl):
                return (
                    (w1h_t, xh_sb, xsl),
                    (w1h_t, xl_sb, xsl),
                    (w1l_t, xh_sb, xsl),
                )

            def emit_mm1_first(hgh, hgl):
                # first 4 h-blocks of (tg0, e0), traced stream-outer and
                # cc-outer across all 8 psum banks: PE's in-order stream
                # consumes each arriving x_hi chunk, then x_lo, then w1l.
                hps8 = [
                    [
                        pp.tile(
                            [128, 512],
                            F32,
                            tag=("h" if hbi < 2 else "y"),
                            bufs=4,
                            name="hps",
                        )
                        for _ in range(TI)
                    ]
                    for hbi in range(4)
                ]
                for si, (w_t, x_t, _) in enumerate(mm1_streams(w1h_first, w1l_first, None)):
                    for cpi in range(CP):
                        cs = slice(2 * cpi, 2 * cpi + 2)
                        for ti in range(TI):
                            for hbi in range(4):
                                nc.tensor.matmul(
                                    hps8[hbi][ti],
                                    w_t[:, cs, hbi * 128 : (hbi + 1) * 128],
                                    x_t[:, cs, ti * 512 : (ti + 1) * 512],
                                    start=(si == 0 and cpi == 0),
                                    stop=(si == 2 and cpi == CP - 1),
                                    perf_mode=DR,
                                )
                for hbi in range(4):
                    emit_gelu_split(0, hbi, hps8[hbi], hgh, hgl)

            def emit_mm1(tg, e, hgh, hgl, hbg_start=0):
                # mm1: h_psum = (x_hi + x_lo) @ w1_hi + x_hi @ w1_lo
                for hbg in range(hbg_start, HB // 4):
                    if tg == 0 and e == 0 and hbg == 1:
                        w1h_t, w1l_t = w1h_second, w1l_second
                    else:
                        hs = slice(hbg * 512, (hbg + 1) * 512)
                        w1h_t = sp.tile([128, CB, 512], F8, tag="w1", bufs=4, name="w1t")
                        nc.sync.dma_start(w1h_t, w1_ap(w1h_d, e, hs))
                        w1l_t = sp.tile([128, CB, 512], F8, tag="w1", bufs=4, name="w1t")
                        nc.sync.dma_start(w1l_t, w1_ap(w1l_d, e, hs))
                    for hbi in range(4):
                        hb = hbg * 4 + hbi
                        hps = [
                            pp.tile([128, 512], F32, tag="h", bufs=4, name="hps")
                            for _ in range(TI)
                        ]
                        for ti in range(TI):
                            gts = slice(tg * TG + ti * 512, tg * TG + (ti + 1) * 512)
                            i = 0
                            for cpi in range(CP):
                                cs = slice(2 * cpi, 2 * cpi + 2)
                                for w_t, x_t, _ in mm1_streams(w1h_t, w1l_t, gts):
                                    nc.tensor.matmul(
                                        hps[ti],
                                        w_t[:, cs, hbi * 128 : (hbi + 1) * 128],
                                        x_t[:, cs, gts],
                                        start=(i == 0),
                                        stop=(i == 3 * CP - 1),
                                        perf_mode=DR,
                                    )
                                    i += 1
                        emit_gelu_split(e, hb, hps, hgh, hgl)

            def emit_mm2(tg, e, hgh, hgl, yac):
                # mm2: y_psum = (hg_hi + hg_lo) @ w2_hi + hg_hi @ w2_lo;
                # then yac (+)= (y_psum + S2*b2) * g_bcast: the psum-reading
                # stt on DVE (GPSIMD can't access PSUM), the SBUF-only
                # cross-expert add on the otherwise-idle Pool engine
                for cb in range(CB):
                    w2h_t = sp.tile([128, HB, 128], F8, tag="w2", bufs=4, name="w2t")
                    w2l_t = sp.tile([128, HB, 128], F8, tag="w2", bufs=4, name="w2t")
                    cbs = slice(cb * 128, (cb + 1) * 128)
                    nc.sync.dma_start(
                        w2h_t, w2h_d[e, :, :].rearrange("(hb p) c -> p hb c", p=128)[:, :, cbs]
                    )
                    nc.sync.dma_start(
                        w2l_t, w2l_d[e, :, :].rearrange("(hb p) c -> p hb c", p=128)[:, :, cbs]
                    )
                    yps = [
                        pp.tile([128, 512], F32, tag="y", bufs=4, name="yps")
                        for _ in range(TI)
                    ]
                    for ti in range(TI):
                        lts = slice(ti * 512, (ti + 1) * 512)
                        i = 0
                        for hpi in range(HP):
                            hs = slice(2 * hpi, 2 * hpi + 2)
                            for w_t, h_t in (
                                (w2h_t, hgh),
                                (w2h_t, hgl),
                                (w2l_t, hgh),
                            ):
                                nc.tensor.matmul(
                                    yps[ti],
                                    w_t[:, hs, :],
                                    h_t[:, hs, lts],
                                    start=(i == 0),
                                    stop=(i == 3 * HP - 1),
                                    perf_mode=DR,
                                )
                                i += 1
                    for ti in range(TI):
                        gts = slice(tg * TG + ti * 512, tg * TG + (ti + 1) * 512)
                        lts = slice(ti * 512, (ti + 1) * 512)
                        if e == 0:
                            nc.vector.scalar_tensor_tensor(
                                out=yac[:, cb, lts],
                                in0=yps[ti],
                                scalar=b2P_sb[:, e, cb : cb + 1],
                                in1=g_bcast_sb[:, e, gts],
                                op0=mybir.AluOpType.add,
                                op1=mybir.AluOpType.mult,
                            )
                        else:
                            yt = sp.tile([128, 512], F32, tag="ytmp", bufs=2, name="yt")
                            nc.vector.scalar_tensor_tensor(
                                out=yt,
                                in0=yps[ti],
                                scalar=b2P_sb[:, e, cb : cb + 1],
                                in1=g_bcast_sb[:, e, gts],
                                op0=mybir.AluOpType.add,
                                op1=mybir.AluOpType.mult,
                            )
                            nc.gpsimd.tensor_tensor(
                                yac[:, cb, lts], yt, yac[:, cb, lts], mybir.AluOpType.add
                            )

            # --- main. Trace order = PE order: the special first block
            # (fills the x/w1 arrival window), the gate prologue (drains on
            # ACT/DVE under the matmul stream), then the expert stream.
            for tg in range(TCG):
                hgh = sp.tile([128, HB, TG], F8, tag="hgh", bufs=1, name="hgh")
                hgl = sp.tile([128, HB, TG], F8, tag="hgl", bufs=1, name="hgl")
                yac = sp.tile([128, CB, TG], F32, tag="yacc", bufs=1, name="yac")
                for e in range(EL):
                    if tg == 0 and e == 0:
                        emit_mm1_first(hgh, hgl)
                        emit_softmax()
                        emit_mm1(tg, e, hgh, hgl, hbg_start=1)
                    else:
                        emit_mm1(tg, e, hgh, hgl)
                    emit_mm2(tg, e, hgh, hgl, yac)
                for cb in range(CB):
                    for ti in range(TI):
                        nc.sync.dma_start(
                            outT_d[
                                cb * 128 : (cb + 1) * 128,
                                tg * TG + ti * 512 : tg * TG + (ti + 1) * 512,
                            ],
                            yac[:, cb, ti * 512 : (ti + 1) * 512],
                        )

    nc.compile()
    return nc


def _get_nc():
    global _CACHED_NC
    if _CACHED_NC is None:
        _CACHED_NC = build_nc()
    return _CACHED_NC


def _split8(a, scale):
    """Dekker 2-term fp8 split of scale*a. Returns (hi, lo) as e4m3."""
    s = (a * scale).astype(np.float32)
    hi = s.astype(NP8)
    lo = (s - hi.astype(np.float32)).astype(NP8)
    return hi, lo


def make_in_maps(x, gate_w, gate_b, w1, b1, w2, b2):
    x = np.asarray(x, np.float32)
    gate_w = np.asarray(gate_w, np.float32)
    gate_b = np.asarray(gate_b, np.float32)
    w1 = np.asarray(w1, np.float32)
    b1 = np.asarray(b1, np.float32)
    w2 = np.asarray(w2, np.float32)
    b2 = np.asarray(b2, np.float32)

    xT = np.ascontiguousarray(x.reshape(N, C).T)
    xh, xl = _split8(xT, 1.0)

    ones32 = np.full((E, EL), S2, np.float16)

    in_maps = []
    for i in range(NCORES):
        lo, hi = EL * i, EL * (i + 1)
        perm = list(range(lo, hi)) + [e for e in range(E) if not (lo <= e < hi)]
        gwh, gwl = _split8(np.ascontiguousarray(gate_w[:, perm]), S1)
        w1h, w1l = _split8(w1[lo:hi], S1)
        w2h, w2l = _split8(w2[lo:hi], S2)
        in_maps.append(
            {
                "xh": xh,
                "xl": xl,
                "gwh": gwh,
                "gwl": gwl,
                "gb": np.ascontiguousarray(gate_b[perm]).reshape(E, 1),
                "ones32": ones32,
                "w1h": w1h,
                "w1l": w1l,
                "b1": np.ascontiguousarray(
                    b1[lo:hi].reshape(EL, HB, 128).transpose(2, 0, 1)
                ),
                "w2h": w2h,
                "w2l": w2l,
                "b2P": np.ascontiguousarray(
                    (S2 * b2[lo:hi]).reshape(EL, CB, 128).transpose(2, 0, 1)
                ),
            }
        )
    return in_maps


def kernel(x, gate_w, gate_b, w1, b1, w2, b2, _trace=False, _tmpdir=None):
    nc = _get_nc()
    in_maps = make_in_maps(x, gate_w, gate_b, w1, b1, w2, b2)
    res = run_bass_kernel_spmd(
        nc,
        in_maps,
        core_ids=list(range(NCORES)),
        trace=_trace,
        tmpdir=_tmpdir,
    )
    acc = res.results[0]["outT"].astype(np.float64)
    for r in res.results[1:]:
        acc += r["outT"]
    out = acc.T.reshape(B, T, C).astype(np.float32)
    if _trace:
        kernel._last_results = res
    return out
